# revision 10
# baseline (speedup 1.0000x reference)
"""GCN dependency-parser kernel for Trainium2 (8 NeuronCores, data-parallel over batch).

Reformulation: the reference's per-batch top-k edge list + PyG GCNConv
scatter/gather is computed densely.  For each batch and GNN layer:
  logits = [h,1] @ bilW @ [d,1]^T            (bilinear attention)
  P      = softmax(logits, axis=-1)
  A[i,j] = P[i,j] if j in top8(row i) else 0
  deg    = 1 + rowsum(A);  dinv = deg^-1/2
  conv(x) = diag(dinv) (A + I) diag(dinv) (x @ W) + b
The masked adjacency is built with a row softmax (exp with accumulated
row sum), the DVE top-8 instruction, and a >=threshold mask - no integer
indices. The aggregation is a dense matmul against the transposed scaled
adjacency.

All matmuls run with fp16 operands (fp32 PSUM accumulation): full PE
rate plus fast-weight-load, and none of the fp32-mode ISA restrictions.
(Historical fp32r notes:
moving free dim even (tokens padded 257->258), stationary free dim even,
and all 4 PE column groups active (output partitions >= 97; the 1-token
tail chunk is padded to M=128 with zeroed lhsT columns).  The bilinear's
per-row bias term (g=500 column of bilW) shifts whole softmax rows, so
it is dropped in the GNN layers and added back only in the final output.)

Layout: features live feature-major [F, T]; the conv feature-matmul
produces token-major g [T, F]; the aggregation matmul (lhsT=g,
rhs=AhatT) flips back to feature-major. Tokens: 257 = 128+128+1 chunks.
"""

import numpy as np

import concourse.bass as bass
import concourse.mybir as mybir
import concourse.tile as tile
from concourse import bacc
from concourse.bass_utils import run_bass_kernel_spmd
from concourse.masks import make_identity

B, S, D = 128, 256, 768
ARC, TAG = 500, 100
LAYERS, TOPK = 2, 8
SP = S + 1                      # 257
SPE = SP + 1                    # 258 (even moving free dim)
FW = 384                        # feats/ut/A free width (token axis, 3*128)
NCORES = 8
BB = B // NCORES                # 16 sentences per core

F32 = mybir.dt.float32
F32R = mybir.dt.float32r
F16 = mybir.dt.float16
AF = mybir.ActivationFunctionType
OP = mybir.AluOpType
AX = mybir.AxisListType

TOK = [(0, 128), (128, 128), (256, 1)]                       # token chunks (start, valid)
ARCC = [(0, 128), (128, 128), (256, 128), (384, 116)]        # arc feature chunks
TAGC = [(0, 100)]
KC6 = [(0, 128), (128, 128), (256, 128), (384, 128), (512, 128), (640, 128)]  # D=768

# output/feature chunks: (key, chunk_idx, start_within_feature, size, base_col_in_concat)
OUTCH = (
    [("ha", i, s, z, 0) for i, (s, z) in enumerate(ARCC)]
    + [("da", i, s, z, 500) for i, (s, z) in enumerate(ARCC)]
    + [("ht", 0, 0, 100, 1000), ("dt", 0, 0, 100, 1100)]
)


def build_program():
    nc = bacc.Bacc("TRN2", target_bir_lowering=False)

    inp_d = nc.declare_dram_parameter("inp", [BB, S, D], F32, isOutput=False)
    sent_d = nc.declare_dram_parameter("sent", [1, D], F32, isOutput=False)
    wall_d = nc.declare_dram_parameter("wall", [D, 1200], F32, isOutput=False)
    ball_d = nc.declare_dram_parameter("ball", [1200, 1], F32, isOutput=False)
    c1aw_d = nc.declare_dram_parameter("c1aw", [ARC, ARC], F32, isOutput=False)
    c2aw_d = nc.declare_dram_parameter("c2aw", [ARC, ARC], F32, isOutput=False)
    c1rw_d = nc.declare_dram_parameter("c1rw", [TAG, TAG], F32, isOutput=False)
    c2rw_d = nc.declare_dram_parameter("c2rw", [TAG, TAG], F32, isOutput=False)
    c1ab_d = nc.declare_dram_parameter("c1ab", [ARC, 1], F32, isOutput=False)
    c2ab_d = nc.declare_dram_parameter("c2ab", [ARC, 1], F32, isOutput=False)
    c1rb_d = nc.declare_dram_parameter("c1rb", [TAG, 1], F32, isOutput=False)
    c2rb_d = nc.declare_dram_parameter("c2rb", [TAG, 1], F32, isOutput=False)
    bilw_d = nc.declare_dram_parameter("bilw", [LAYERS + 1, ARC + 1, ARC + 1], F32, isOutput=False)
    w4_d = nc.declare_dram_parameter("w4", [ARC, 2], F32, isOutput=False)   # bilW[2][:500,500]
    bb2_d = nc.declare_dram_parameter("bb2", [128, 1], F32, isOutput=False)  # bil_b[2]+bilW[2][500,500]
    bwbc_d = nc.declare_dram_parameter("bwbc", [LAYERS + 1, ARC + 1, 1], F32, isOutput=False)

    arc_d = nc.declare_dram_parameter("arc", [BB, SP, SP], F32, isOutput=True)
    ht_d = nc.declare_dram_parameter("ht", [BB, SP, TAG], F32, isOutput=True)
    dt_d = nc.declare_dram_parameter("dt", [BB, SP, TAG], F32, isOutput=True)

    with tile.TileContext(nc) as tc:
        with (
            tc.tile_pool(name="consts", bufs=1) as consts,
            tc.tile_pool(name="stage", bufs=1) as stage,
            tc.tile_pool(name="xtok", bufs=3) as xtok_p,
            tc.tile_pool(name="xT", bufs=8) as xT_p,
            tc.tile_pool(name="feats", bufs=22) as feats_p,
            tc.tile_pool(name="g", bufs=12) as g_p,
            tc.tile_pool(name="tmp", bufs=4) as tmp_p,
            tc.tile_pool(name="soft", bufs=4) as soft_p,
            tc.tile_pool(name="ahat", bufs=6) as ahat_p,
            tc.tile_pool(name="ut", bufs=6) as ut_p,
            tc.tile_pool(name="tiny", bufs=32) as tiny_p,
            tc.tile_pool(name="ostage", bufs=6) as ost_p,
            tc.tile_pool(name="ps", bufs=7, space="PSUM") as ps_p,
        ):
            ident = consts.tile([128, 128], F32, name="ident", tag="ident")
            make_identity(nc, ident[:])

            def round_in(dram_ap, p, f, tag):
                """DMA fp32 DRAM -> SBUF, then DVE-round into an fp32r tile."""
                st = stage.tile([128, 1200], F32, name="stage", tag="stage")
                nc.sync.dma_start(out=st[0:p, 0:f], in_=dram_ap)
                t = consts.tile([p, f], F16, name=tag, tag=tag)
                nc.vector.tensor_copy(t[:, :], st[0:p, 0:f])
                return t

            wall = [round_in(wall_d[s:s + z, :], z, 1200, f"wall{i}")
                    for i, (s, z) in enumerate(KC6)]
            c1aw = [round_in(c1aw_d[s:s + z, :], z, ARC, f"c1aw{i}")
                    for i, (s, z) in enumerate(ARCC)]
            c2aw = [round_in(c2aw_d[s:s + z, :], z, ARC, f"c2aw{i}")
                    for i, (s, z) in enumerate(ARCC)]
            c1rw = [round_in(c1rw_d[0:100, :], 100, TAG, "c1rw")]
            c2rw = [round_in(c2rw_d[0:100, :], 100, TAG, "c2rw")]
            bw = [[round_in(bilw_d[k, s:s + z, :], z, ARC + 1, f"bw{k}_{i}")
                   for i, (s, z) in enumerate(ARCC)] for k in range(LAYERS + 1)]
            w4 = [round_in(w4_d[s:s + z, :], z, 2, f"w4_{i}")
                  for i, (s, z) in enumerate(ARCC)]
            zcst = consts.tile([128, SPE], F32, name="zcst", tag="zcst")
            nc.vector.memset(zcst[:, :], 0.0)
            ident16 = consts.tile([128, 128], F16, name="ident16", tag="ident16")
            nc.vector.tensor_copy(ident16[:, :], ident[:, :])

            def bias_in(dram_ap, p, tag):
                t = consts.tile([p, 1], F32, name=tag, tag=tag)
                nc.sync.dma_start(out=t[:, :], in_=dram_ap)
                return t

            ballc = {}
            for key, ci, fs, fz, base in OUTCH:
                ballc[(key, ci)] = bias_in(ball_d[base + fs:base + fs + fz, :], fz, f"ball{key}{ci}")
            c1b = {"ha": [bias_in(c1ab_d[s:s + z, :], z, f"c1ab{i}") for i, (s, z) in enumerate(ARCC)],
                   "ht": [bias_in(c1rb_d[0:100, :], 100, "c1rb")]}
            c1b["da"] = c1b["ha"]
            c1b["dt"] = c1b["ht"]
            c2b = {"ha": [bias_in(c2ab_d[s:s + z, :], z, f"c2ab{i}") for i, (s, z) in enumerate(ARCC)],
                   "ht": [bias_in(c2rb_d[0:100, :], 100, "c2rb")]}
            c2b["da"] = c2b["ha"]
            c2b["dt"] = c2b["ht"]
            bb2 = bias_in(bb2_d[:, :], 128, "bb2")
            bwbc = [[bias_in(bwbc_d[k, s:s + z, :], z, f"bwbc{k}_{i}")
                     for i, (s, z) in enumerate(ARCC)] for k in range(LAYERS + 1)]

            wsets = {"ha": (c1aw, c2aw, ARCC, ARC), "da": (c1aw, c2aw, ARCC, ARC),
                     "ht": (c1rw, c2rw, TAGC, TAG), "dt": (c1rw, c2rw, TAGC, TAG)}

            def elu_epilogue(psum_ap, bias_ap, out_ap, fz):
                """out = elu(psum + bias); aps cover [fz, 0:257]; out fp32r."""
                u = tmp_p.tile([fz, SP], F32, name="tmp_u", tag="tmp_u")
                nc.scalar.activation(u[:, :], psum_ap, AF.Relu, bias=bias_ap)
                m = tmp_p.tile([fz, SP], F32, name="tmp_m", tag="tmp_m")
                nc.vector.tensor_scalar(m[:, :], psum_ap, bias_ap, 0.0, op0=OP.add, op1=OP.min)
                v = tmp_p.tile([fz, SP], F32, name="tmp_v", tag="tmp_v")
                nc.scalar.activation(v[:, :], m[:, :], AF.Exp)
                w = tmp_p.tile([fz, SP], F32, name="tmp_w", tag="tmp_w")
                nc.gpsimd.tensor_scalar_add(w[:, :], v[:, :], -1.0)
                nc.gpsimd.tensor_add(out_ap, w[:, :], u[:, :])

            def new_feats():
                f = {}
                for key in ("ha", "da"):
                    f[key] = [feats_p.tile([z, FW], F16, name="feats", tag="feats")
                              for (s, z) in ARCC]
                for key in ("ht", "dt"):
                    f[key] = [feats_p.tile([100, FW], F16, name="feats", tag="feats")]
                # pad token columns (257:384) must be exact zeros: they feed the
                # padded M=128 matmuls of the 1-token tail chunk.
                for key in ("ha", "da", "ht", "dt"):
                    for t in f[key]:
                        z = t.shape[0]
                        nc.gpsimd.tensor_copy(t[0:z, SP:FW], zcst[0:z, 0:FW - SP])
                return f

            def bilinear(k, f):
                """Returns 3 psum tiles of logits [128, 258] (token chunks; only
                [0:tz, 0:257] valid). The g=500 per-row-constant term is omitted
                (softmax-invariant)."""
                ha, da = f["ha"], f["da"]
                ut = []
                for gi, (gs, gz) in enumerate(ARCC):
                    psu = ps_p.tile([gz, SPE], F32, name="ps", tag="ps")
                    for ei, (es, ez) in enumerate(ARCC):
                        nc.tensor.matmul(psu[:, :], bw[k][ei][:, gs:gs + gz], ha[ei][:, 0:SPE],
                                         start=(ei == 0), stop=(ei == 3))
                    u = ut_p.tile([gz, FW], F16, name="ut", tag="ut")
                    nc.vector.tensor_scalar_add(u[:, 0:SPE], psu[:, :], bwbc[k][gi][:, :])
                    nc.gpsimd.tensor_copy(u[:, SPE:FW], zcst[0:u.shape[0], 0:FW - SPE])
                    ut.append(u)
                lps = []
                for ti, (ts, tz) in enumerate(TOK):
                    psl = ps_p.tile([128, SPE], F32, name="ps", tag="ps")
                    for gi, (gs, gz) in enumerate(ARCC):
                        nc.tensor.matmul(psl[:, :], ut[gi][:, ts:ts + 128], da[gi][:, 0:SPE],
                                         start=(gi == 0), stop=(gi == 3))
                    lps.append(psl)
                return lps

            def build_ahatT(lps):
                """softmax + top8 mask + scalings + transpose -> AhatT tiles [j, i]."""
                A_t = []
                es = []
                t8s = []
                Z_all = tiny_p.tile([128, 3], F32, name="tinyZ", tag="tinyZ")
                s8_all = tiny_p.tile([128, 3], F32, name="tinyZ", tag="tinyZ")
                for ti, (ts, tz) in enumerate(TOK):
                    psl = lps[ti]
                    nm = tiny_p.tile([tz, 1], F32, name="tiny", tag="tiny")
                    nc.vector.tensor_reduce(nm[:, :], psl[0:tz, 0:SP], axis=AX.X,
                                            op=OP.max, negate=True)
                    e = soft_p.tile([tz, SP], F32, name="soft_e", tag="soft_e")
                    nc.scalar.activation(e[:, :], psl[0:tz, 0:SP], AF.Exp,
                                         bias=nm[:, :], accum_out=Z_all[0:tz, ti:ti + 1])
                    t8 = tiny_p.tile([tz, 8], F32, name="tiny8", tag="tiny8")
                    nc.vector.max(t8[:, :], e[:, :])
                    nc.vector.tensor_reduce(s8_all[0:tz, ti:ti + 1], t8[:, :],
                                            axis=AX.X, op=OP.add)
                    es.append(e)
                    t8s.append(t8)
                # deg = (Z + s8)/Z ; dinv = sqrt(Z/(Z+s8)) ; rowfac = dinv/Z
                zp = tiny_p.tile([128, 3], F32, name="tinyZ", tag="tinyZ")
                nc.vector.tensor_add(zp[:, :], Z_all[:, :], s8_all[:, :])
                r = tiny_p.tile([128, 3], F32, name="tinyZ", tag="tinyZ")
                nc.vector.reciprocal(r[:, :], zp[:, :])
                q = tiny_p.tile([128, 3], F32, name="tinyZ", tag="tinyZ")
                nc.vector.tensor_mul(q[:, :], r[:, :], Z_all[:, :])
                dv_all = tiny_p.tile([128, 3], F32, name="tinyZ", tag="tinyZ")
                nc.scalar.sqrt(dv_all[:, :], q[:, :])
                rz = tiny_p.tile([128, 3], F32, name="tinyZ", tag="tinyZ")
                nc.vector.reciprocal(rz[:, :], Z_all[:, :])
                rf_all = tiny_p.tile([128, 3], F32, name="tinyZ", tag="tinyZ")
                nc.vector.tensor_mul(rf_all[:, :], dv_all[:, :], rz[:, :])
                for ti, (ts, tz) in enumerate(TOK):
                    e, t8 = es[ti], t8s[ti]
                    mask = soft_p.tile([tz, SP], F32, name="soft_mask", tag="soft_mask")
                    nc.vector.tensor_scalar(mask[:, :], e[:, :], t8[:, 7:8], None, op0=OP.is_ge)
                    A = soft_p.tile([max(tz, 2), FW], F32, name="soft_A", tag="soft_A")
                    nc.vector.scalar_tensor_tensor(A[0:tz, 0:SP], e[:, :],
                                                   rf_all[0:tz, ti:ti + 1], mask[:, :],
                                                   op0=OP.mult, op1=OP.mult)
                    # diag: A[p, 128*ti + p] += dinv[p]  (column scaling comes after transpose)
                    dsl = A[0:tz, 128 * ti:128 * ti + tz]
                    nc.vector.scalar_tensor_tensor(dsl, ident[0:tz, 0:tz],
                                                   dv_all[0:tz, ti:ti + 1], dsl,
                                                   op0=OP.mult, op1=OP.add)
                    A_t.append(A)
                ahatT = []
                for jc in range(3):
                    pst = ps_p.tile([128, SPE], F32, name="ps", tag="ps")
                    for ti, (ts, tz) in enumerate(TOK):
                        tze = tz if tz > 1 else 2
                        nc.tensor.transpose(pst[:, ts:ts + tze],
                                            A_t[ti][0:tze, 128 * jc:128 * jc + 128],
                                            ident[0:tze, 0:tze])
                    ah = ahat_p.tile([128, SPE], F16, name="ahat", tag="ahat")
                    if jc < 2:
                        nc.vector.tensor_scalar_mul(ah[:, :], pst[:, :], dv_all[:, jc:jc + 1])
                    else:
                        nc.gpsimd.tensor_copy(ah[:, :], zcst[:, :])
                        nc.vector.tensor_scalar_mul(ah[0:1, 0:SP], pst[0:1, 0:SP],
                                                    dv_all[0:1, 2:3])
                    ahatT.append(ah)
                return ahatT

            def conv(f_in, widx, ahatT, elu, bset):
                """One GCNConv on all four features. widx 0 -> c1 weights, 1 -> c2."""
                g = {}
                for key in ("ha", "da", "ht", "dt"):
                    w1, w2, chunks, width = wsets[key]
                    W = (w1, w2)[widx]
                    gl = []
                    for ti, (ts, tz) in enumerate(TOK):
                        psg = ps_p.tile([128, width], F32, name="ps", tag="ps")
                        nchunk = len(chunks)
                        for fi, (fs, fz) in enumerate(chunks):
                            nc.tensor.matmul(psg[:, :], f_in[key][fi][:, ts:ts + 128],
                                             W[fi][:, :], start=(fi == 0),
                                             stop=(fi == nchunk - 1))
                        gt = g_p.tile([128, width], F16, name="g", tag="g")
                        nc.vector.tensor_copy(gt[:, :], psg[:, :])
                        gl.append(gt)
                    g[key] = gl
                f_out = new_feats()
                for key, ci, fs, fz, _ in OUTCH:
                    psh = ps_p.tile([fz, SPE], F32, name="ps", tag="ps")
                    for jc in range(3):
                        nc.tensor.matmul(psh[:, :], g[key][jc][:, fs:fs + fz],
                                         ahatT[jc][:, :], start=(jc == 0), stop=(jc == 2))
                    bias_ap = bset[key][ci][:, :]
                    if elu:
                        elu_epilogue(psh[:, 0:SP], bias_ap, f_out[key][ci][0:fz, 0:SP], fz)
                    else:
                        nc.scalar.activation(f_out[key][ci][0:fz, 0:SP], psh[:, 0:SP],
                                             AF.Identity, bias=bias_ap)
                return f_out

            for b in range(BB):
                # ---- load x token-major, transpose to feature-major x^T ----
                xs = []
                for ti, (ts, tz) in enumerate(TOK):
                    xt = xtok_p.tile([max(tz, 2), D], F32, name="xtok", tag="xtok")
                    if ti == 0:
                        nc.sync.dma_start(out=xt[0:1, :], in_=sent_d[:, :])
                        nc.sync.dma_start(out=xt[1:128, :], in_=inp_d[b, 0:127, :])
                    else:
                        nc.sync.dma_start(out=xt[0:tz, :], in_=inp_d[b, ts - 1:ts - 1 + tz, :])
                    xs.append(xt)
                xT = []
                for fc, (fs, fz) in enumerate(KC6):
                    x_f = xT_p.tile([128, SPE], F16, name="xT", tag="xT")
                    for ti, (ts, tz) in enumerate(TOK):
                        tze = tz if tz > 1 else 2
                        pst = ps_p.tile([128, tze], F32, name="ps", tag="ps")
                        nc.tensor.transpose(pst[:, :], xs[ti][0:tze, fs:fs + fz],
                                            ident[0:tze, 0:tze])
                        nc.vector.tensor_copy(x_f[:, ts:ts + tze], pst[:, :])
                    xT.append(x_f)

                # ---- initial projection + elu -> feature-major feats ----
                feats = new_feats()
                for key, ci, fs, fz, base in OUTCH:
                    psp = ps_p.tile([fz, SPE], F32, name="ps", tag="ps")
                    for kc, (ks, kz) in enumerate(KC6):
                        nc.tensor.matmul(psp[:, :], wall[kc][:, base + fs:base + fs + fz],
                                         xT[kc][:, :], start=(kc == 0), stop=(kc == 5))
                    elu_epilogue(psp[:, 0:SP], ballc[(key, ci)][:, :],
                                 feats[key][ci][0:fz, 0:SP], fz)

                # ---- GNN layers ----
                for k in range(LAYERS):
                    lps = bilinear(k, feats)
                    ahatT = build_ahatT(lps)
                    e1 = conv(feats, 0, ahatT, elu=True, bset=c1b)
                    feats = conv(e1, 1, ahatT, elu=False, bset=c2b)

                # ---- final bilinear: logits + u4 (g=500 row term) + bil_b[2] ----
                lps = bilinear(LAYERS, feats)
                u4ps = []
                for ti, (ts, tz) in enumerate(TOK):
                    ps4 = ps_p.tile([128, 2], F32, name="ps", tag="ps")
                    for ei, (es, ez) in enumerate(ARCC):
                        nc.tensor.matmul(ps4[:, :], feats["ha"][ei][:, ts:ts + 128],
                                         w4[ei][:, :], start=(ei == 0), stop=(ei == 3))
                    u4ps.append(ps4)
                for ti, (ts, tz) in enumerate(TOK):
                    u4b = tiny_p.tile([tz, 1], F32, name="tiny", tag="tiny")
                    nc.vector.tensor_add(u4b[:, :], u4ps[ti][0:tz, 0:1], bb2[0:tz, :])
                    ob = ost_p.tile([tz, SP], F32, name="ostage", tag="ostage")
                    nc.scalar.activation(ob[:, :], lps[ti][0:tz, 0:SP], AF.Identity,
                                         bias=u4b[:, :])
                    nc.sync.dma_start(out=arc_d[b, ts:ts + tz, :], in_=ob[:, :])

                # ---- tag outputs: transpose feature-major -> token-major ----
                for key, dram in (("ht", ht_d), ("dt", dt_d)):
                    src = feats[key][0]
                    for ti, (ts, tz) in enumerate(TOK):
                        pst = ps_p.tile([128, TAG], F16, name="ps", tag="ps")
                        nc.tensor.transpose(pst[:, :], src[:, ts:ts + 128],
                                            ident16[0:100, 0:100])
                        ob = ost_p.tile([tz, TAG], F32, name="ostage", tag="ostage")
                        nc.vector.tensor_copy(ob[:, :], pst[0:tz, :])
                        nc.sync.dma_start(out=dram[b, ts:ts + tz, :], in_=ob[:, :])

    nc.finalize()
    return nc


_PROGRAM = None


def kernel(_run_kwargs=None, **inputs):
    global _PROGRAM
    run_kwargs = _run_kwargs or {}
    inp = np.asarray(inputs["inputs"], np.float32)
    sent = np.asarray(inputs["head_sentinel"], np.float32).reshape(1, D)
    wall = np.concatenate(
        [np.asarray(inputs[k], np.float32) for k in ("W_ha", "W_da", "W_ht", "W_dt")], axis=1
    )
    ball = np.concatenate(
        [np.asarray(inputs[k], np.float32) for k in ("b_ha", "b_da", "b_ht", "b_dt")]
    ).reshape(1200, 1)
    bilw = np.asarray(inputs["bil_W"], np.float32)
    bilb = np.asarray(inputs["bil_b"], np.float32)
    w4 = np.zeros((ARC, 2), np.float32)
    w4[:, 0] = bilw[LAYERS, 0:ARC, ARC]
    params = {
        "sent": sent, "wall": wall, "ball": ball, "bilw": bilw, "w4": w4,
        "c1aw": np.asarray(inputs["c1a_W"], np.float32),
        "c2aw": np.asarray(inputs["c2a_W"], np.float32),
        "c1rw": np.asarray(inputs["c1r_W"], np.float32),
        "c2rw": np.asarray(inputs["c2r_W"], np.float32),
        "c1ab": np.asarray(inputs["c1a_b"], np.float32).reshape(ARC, 1),
        "c2ab": np.asarray(inputs["c2a_b"], np.float32).reshape(ARC, 1),
        "c1rb": np.asarray(inputs["c1r_b"], np.float32).reshape(TAG, 1),
        "c2rb": np.asarray(inputs["c2r_b"], np.float32).reshape(TAG, 1),
        "bb2": np.full((128, 1), bilb[LAYERS] + bilw[LAYERS, ARC, ARC], np.float32),
        "bwbc": np.ascontiguousarray(bilw[:, ARC, :, None]),
    }

    if _PROGRAM is None:
        _PROGRAM = build_program()
    nc = _PROGRAM

    in_maps = [dict(params, inp=np.ascontiguousarray(inp[c * BB:(c + 1) * BB]))
               for c in range(NCORES)]
    res = run_bass_kernel_spmd(nc, in_maps, list(range(NCORES)), **run_kwargs)
    arc = np.concatenate([r["arc"] for r in res.results], axis=0)
    ht = np.concatenate([r["ht"] for r in res.results], axis=0)
    dt = np.concatenate([r["dt"] for r in res.results], axis=0)
    if run_kwargs:
        return (arc, ht, dt), res
    return arc, ht, dt


# revision 11
# speedup vs baseline: 2.1037x; 2.1037x over previous
"""GCN dependency-parser kernel for Trainium2 (8 NeuronCores, data-parallel over batch).

Reformulation: the reference's per-batch top-k edge list + PyG GCNConv
scatter/gather is computed densely.  For each batch and GNN layer:
  logits = [h,1] @ bilW @ [d,1]^T            (bilinear attention)
  P      = softmax(logits, axis=-1)
  A[i,j] = P[i,j] if j in top8(row i) else 0
  deg    = 1 + rowsum(A);  dinv = deg^-1/2
  conv(x) = diag(dinv) (A + I) diag(dinv) (x @ W) + b
The masked adjacency is built with a row softmax (exp with accumulated
row sum), the DVE top-8 instruction, and a >=threshold mask - no integer
indices. The aggregation is a dense matmul against the transposed scaled
adjacency.

All matmuls run with fp16 operands (fp32 PSUM accumulation): full PE
rate plus fast-weight-load, and none of the fp32-mode ISA restrictions.
(Historical fp32r notes:
moving free dim even (tokens padded 257->258), stationary free dim even,
and all 4 PE column groups active (output partitions >= 97; the 1-token
tail chunk is padded to M=128 with zeroed lhsT columns).  The bilinear's
per-row bias term (g=500 column of bilW) shifts whole softmax rows, so
it is dropped in the GNN layers and added back only in the final output.)

Layout: features live feature-major [F, T]; the conv feature-matmul
produces token-major g [T, F]; the aggregation matmul (lhsT=g,
rhs=AhatT) flips back to feature-major. Tokens: 257 = 128+128+1 chunks.
"""

import numpy as np

import concourse.bass as bass
import concourse.mybir as mybir
import concourse.tile as tile
from concourse import bacc
from concourse.bass_utils import run_bass_kernel_spmd
from concourse.masks import make_identity

B, S, D = 128, 256, 768
ARC, TAG = 500, 100
LAYERS, TOPK = 2, 8
SP = S + 1                      # 257
SPE = SP + 1                    # 258 (even moving free dim)
FW = 384                        # feats/ut/A free width (token axis, 3*128)
NCORES = 8
BB = B // NCORES                # 16 sentences per core

F32 = mybir.dt.float32
F32R = mybir.dt.float32r
F16 = mybir.dt.float16
AF = mybir.ActivationFunctionType
OP = mybir.AluOpType
AX = mybir.AxisListType

TOK = [(0, 128), (128, 128), (256, 1)]                       # token chunks (start, valid)
ARCC = [(0, 128), (128, 128), (256, 128), (384, 116)]        # arc feature chunks
TAGC = [(0, 100)]
KC6 = [(0, 128), (128, 128), (256, 128), (384, 128), (512, 128), (640, 128)]  # D=768

# output/feature chunks: (key, chunk_idx, start_within_feature, size, base_col_in_concat)
OUTCH = (
    [("ha", i, s, z, 0) for i, (s, z) in enumerate(ARCC)]
    + [("da", i, s, z, 500) for i, (s, z) in enumerate(ARCC)]
    + [("ht", 0, 0, 100, 1000), ("dt", 0, 0, 100, 1100)]
)


def build_program():
    nc = bacc.Bacc("TRN2", target_bir_lowering=False)

    inp_d = nc.declare_dram_parameter("inp", [BB, S, D], F32, isOutput=False)
    sent_d = nc.declare_dram_parameter("sent", [1, D], F32, isOutput=False)
    wall_d = nc.declare_dram_parameter("wall", [D, 1200], F32, isOutput=False)
    ball_d = nc.declare_dram_parameter("ball", [1200, 1], F32, isOutput=False)
    c1aw_d = nc.declare_dram_parameter("c1aw", [ARC, ARC], F32, isOutput=False)
    c2aw_d = nc.declare_dram_parameter("c2aw", [ARC, ARC], F32, isOutput=False)
    c1rw_d = nc.declare_dram_parameter("c1rw", [TAG, TAG], F32, isOutput=False)
    c2rw_d = nc.declare_dram_parameter("c2rw", [TAG, TAG], F32, isOutput=False)
    c1ab_d = nc.declare_dram_parameter("c1ab", [ARC, 1], F32, isOutput=False)
    c2ab_d = nc.declare_dram_parameter("c2ab", [ARC, 1], F32, isOutput=False)
    c1rb_d = nc.declare_dram_parameter("c1rb", [TAG, 1], F32, isOutput=False)
    c2rb_d = nc.declare_dram_parameter("c2rb", [TAG, 1], F32, isOutput=False)
    bilw_d = nc.declare_dram_parameter("bilw", [LAYERS + 1, ARC + 1, ARC + 1], F32, isOutput=False)
    w4_d = nc.declare_dram_parameter("w4", [ARC, 2], F32, isOutput=False)   # bilW[2][:500,500]
    bb2_d = nc.declare_dram_parameter("bb2", [128, 1], F32, isOutput=False)  # bil_b[2]+bilW[2][500,500]
    bwbc_d = nc.declare_dram_parameter("bwbc", [LAYERS + 1, ARC + 1, 1], F32, isOutput=False)

    arc_d = nc.declare_dram_parameter("arc", [BB, SP, SP], F32, isOutput=True)
    ht_d = nc.declare_dram_parameter("ht", [BB, SP, TAG], F32, isOutput=True)
    dt_d = nc.declare_dram_parameter("dt", [BB, SP, TAG], F32, isOutput=True)

    with tile.TileContext(nc) as tc:
        with (
            tc.tile_pool(name="consts", bufs=1) as consts,
            tc.tile_pool(name="stage", bufs=1) as stage,
            tc.tile_pool(name="xtok", bufs=3) as xtok_p,
            tc.tile_pool(name="xT", bufs=8) as xT_p,
            tc.tile_pool(name="feats", bufs=22) as feats_p,
            tc.tile_pool(name="g", bufs=12) as g_p,
            tc.tile_pool(name="tmp", bufs=4) as tmp_p,
            tc.tile_pool(name="soft", bufs=4) as soft_p,
            tc.tile_pool(name="ahat", bufs=6) as ahat_p,
            tc.tile_pool(name="ut", bufs=6) as ut_p,
            tc.tile_pool(name="tiny", bufs=32) as tiny_p,
            tc.tile_pool(name="ostage", bufs=6) as ost_p,
            tc.tile_pool(name="ps", bufs=7, space="PSUM") as ps_p,
        ):
            ident = consts.tile([128, 128], F32, name="ident", tag="ident")
            make_identity(nc, ident[:])

            def round_in(dram_ap, p, f, tag):
                """DMA fp32 DRAM -> SBUF, then DVE-round into an fp32r tile."""
                st = stage.tile([128, 1200], F32, name="stage", tag="stage")
                nc.sync.dma_start(out=st[0:p, 0:f], in_=dram_ap)
                t = consts.tile([p, f], F16, name=tag, tag=tag)
                nc.vector.tensor_copy(t[:, :], st[0:p, 0:f])
                return t

            wall = [round_in(wall_d[s:s + z, :], z, 1200, f"wall{i}")
                    for i, (s, z) in enumerate(KC6)]
            c1aw = [round_in(c1aw_d[s:s + z, :], z, ARC, f"c1aw{i}")
                    for i, (s, z) in enumerate(ARCC)]
            c2aw = [round_in(c2aw_d[s:s + z, :], z, ARC, f"c2aw{i}")
                    for i, (s, z) in enumerate(ARCC)]
            c1rw = [round_in(c1rw_d[0:100, :], 100, TAG, "c1rw")]
            c2rw = [round_in(c2rw_d[0:100, :], 100, TAG, "c2rw")]
            bw = [[round_in(bilw_d[k, s:s + z, :], z, ARC + 1, f"bw{k}_{i}")
                   for i, (s, z) in enumerate(ARCC)] for k in range(LAYERS + 1)]
            w4 = [round_in(w4_d[s:s + z, :], z, 2, f"w4_{i}")
                  for i, (s, z) in enumerate(ARCC)]
            zcst = consts.tile([128, SPE], F32, name="zcst", tag="zcst")
            nc.vector.memset(zcst[:, :], 0.0)
            ident16 = consts.tile([128, 128], F16, name="ident16", tag="ident16")
            nc.vector.tensor_copy(ident16[:, :], ident[:, :])

            def bias_in(dram_ap, p, tag):
                t = consts.tile([p, 1], F32, name=tag, tag=tag)
                nc.sync.dma_start(out=t[:, :], in_=dram_ap)
                return t

            ballc = {}
            for key, ci, fs, fz, base in OUTCH:
                ballc[(key, ci)] = bias_in(ball_d[base + fs:base + fs + fz, :], fz, f"ball{key}{ci}")
            c1b = {"ha": [bias_in(c1ab_d[s:s + z, :], z, f"c1ab{i}") for i, (s, z) in enumerate(ARCC)],
                   "ht": [bias_in(c1rb_d[0:100, :], 100, "c1rb")]}
            c1b["da"] = c1b["ha"]
            c1b["dt"] = c1b["ht"]
            c2b = {"ha": [bias_in(c2ab_d[s:s + z, :], z, f"c2ab{i}") for i, (s, z) in enumerate(ARCC)],
                   "ht": [bias_in(c2rb_d[0:100, :], 100, "c2rb")]}
            c2b["da"] = c2b["ha"]
            c2b["dt"] = c2b["ht"]
            bb2 = bias_in(bb2_d[:, :], 128, "bb2")
            bwbc = [[bias_in(bwbc_d[k, s:s + z, :], z, f"bwbc{k}_{i}")
                     for i, (s, z) in enumerate(ARCC)] for k in range(LAYERS + 1)]

            wsets = {"ha": (c1aw, c2aw, ARCC, ARC), "da": (c1aw, c2aw, ARCC, ARC),
                     "ht": (c1rw, c2rw, TAGC, TAG), "dt": (c1rw, c2rw, TAGC, TAG)}

            def elu_epilogue(psum_ap, bias_ap, out_ap, fz):
                """out = elu(psum + bias); aps cover [fz, 0:257]; out fp32r."""
                u = tmp_p.tile([fz, SP], F32, name="tmp_u", tag="tmp_u")
                nc.scalar.activation(u[:, :], psum_ap, AF.Relu, bias=bias_ap)
                m = tmp_p.tile([fz, SP], F32, name="tmp_m", tag="tmp_m")
                nc.vector.tensor_scalar(m[:, :], psum_ap, bias_ap, 0.0, op0=OP.add, op1=OP.min)
                v = tmp_p.tile([fz, SP], F32, name="tmp_v", tag="tmp_v")
                nc.scalar.activation(v[:, :], m[:, :], AF.Exp)
                nc.vector.scalar_tensor_tensor(out_ap, v[:, :], -1.0, u[:, :],
                                               op0=OP.add, op1=OP.add)

            def new_feats():
                f = {}
                for key in ("ha", "da"):
                    f[key] = [feats_p.tile([z, FW], F16, name="feats", tag="feats")
                              for (s, z) in ARCC]
                for key in ("ht", "dt"):
                    f[key] = [feats_p.tile([100, FW], F16, name="feats", tag="feats")]
                # pad token columns (257:384) must be exact zeros: they feed the
                # padded M=128 matmuls of the 1-token tail chunk.
                for key in ("ha", "da", "ht", "dt"):
                    for t in f[key]:
                        z = t.shape[0]
                        nc.gpsimd.tensor_copy(t[0:z, SP:FW], zcst[0:z, 0:FW - SP])
                return f

            def bilinear(k, f):
                """Returns 3 psum tiles of logits [128, 258] (token chunks; only
                [0:tz, 0:257] valid). The g=500 per-row-constant term is omitted
                (softmax-invariant)."""
                ha, da = f["ha"], f["da"]
                ut = []
                for gi, (gs, gz) in enumerate(ARCC):
                    psu = ps_p.tile([gz, SPE], F32, name="ps", tag="ps")
                    for ei, (es, ez) in enumerate(ARCC):
                        nc.tensor.matmul(psu[:, :], bw[k][ei][:, gs:gs + gz], ha[ei][:, 0:SPE],
                                         start=(ei == 0), stop=(ei == 3))
                    u = ut_p.tile([gz, FW], F16, name="ut", tag="ut")
                    nc.vector.tensor_scalar_add(u[:, 0:SPE], psu[:, :], bwbc[k][gi][:, :])
                    nc.gpsimd.tensor_copy(u[:, SPE:FW], zcst[0:u.shape[0], 0:FW - SPE])
                    ut.append(u)
                lps = []
                for ti, (ts, tz) in enumerate(TOK):
                    psl = ps_p.tile([128, SPE], F32, name="ps", tag="ps")
                    for gi, (gs, gz) in enumerate(ARCC):
                        nc.tensor.matmul(psl[:, :], ut[gi][:, ts:ts + 128], da[gi][:, 0:SPE],
                                         start=(gi == 0), stop=(gi == 3))
                    lps.append(psl)
                return lps

            def build_ahatT(lps):
                """softmax + top8 mask + scalings + transpose -> AhatT tiles [j, i]."""
                A_t = []
                es = []
                t8s = []
                Z_all = tiny_p.tile([128, 3], F32, name="tinyZ", tag="tinyZ")
                s8_all = tiny_p.tile([128, 3], F32, name="tinyZ", tag="tinyZ")
                for ti, (ts, tz) in enumerate(TOK):
                    psl = lps[ti]
                    nm = tiny_p.tile([tz, 1], F32, name="tiny", tag="tiny")
                    nc.vector.tensor_reduce(nm[:, :], psl[0:tz, 0:SP], axis=AX.X,
                                            op=OP.max, negate=True)
                    e = soft_p.tile([tz, SP], F32, name="soft_e", tag="soft_e")
                    nc.scalar.activation(e[:, :], psl[0:tz, 0:SP], AF.Exp,
                                         bias=nm[:, :], accum_out=Z_all[0:tz, ti:ti + 1])
                    t8 = tiny_p.tile([tz, 8], F32, name="tiny8", tag="tiny8")
                    nc.vector.max(t8[:, :], e[:, :])
                    nc.vector.tensor_reduce(s8_all[0:tz, ti:ti + 1], t8[:, :],
                                            axis=AX.X, op=OP.add)
                    es.append(e)
                    t8s.append(t8)
                # deg = (Z + s8)/Z ; dinv = sqrt(Z/(Z+s8)) ; rowfac = dinv/Z
                zp = tiny_p.tile([128, 3], F32, name="tinyZ", tag="tinyZ")
                nc.vector.tensor_add(zp[:, :], Z_all[:, :], s8_all[:, :])
                r = tiny_p.tile([128, 3], F32, name="tinyZ", tag="tinyZ")
                nc.vector.reciprocal(r[:, :], zp[:, :])
                q = tiny_p.tile([128, 3], F32, name="tinyZ", tag="tinyZ")
                nc.vector.tensor_mul(q[:, :], r[:, :], Z_all[:, :])
                dv_all = tiny_p.tile([128, 3], F32, name="tinyZ", tag="tinyZ")
                nc.scalar.sqrt(dv_all[:, :], q[:, :])
                rz = tiny_p.tile([128, 3], F32, name="tinyZ", tag="tinyZ")
                nc.vector.reciprocal(rz[:, :], Z_all[:, :])
                rf_all = tiny_p.tile([128, 3], F32, name="tinyZ", tag="tinyZ")
                nc.vector.tensor_mul(rf_all[:, :], dv_all[:, :], rz[:, :])
                for ti, (ts, tz) in enumerate(TOK):
                    e, t8 = es[ti], t8s[ti]
                    mask = soft_p.tile([tz, SP], F32, name="soft_mask", tag="soft_mask")
                    nc.vector.tensor_scalar(mask[:, :], e[:, :], t8[:, 7:8], None, op0=OP.is_ge)
                    A = soft_p.tile([max(tz, 2), FW], F32, name="soft_A", tag="soft_A")
                    nc.vector.scalar_tensor_tensor(A[0:tz, 0:SP], e[:, :],
                                                   rf_all[0:tz, ti:ti + 1], mask[:, :],
                                                   op0=OP.mult, op1=OP.mult)
                    # diag: A[p, 128*ti + p] += dinv[p]  (column scaling comes after transpose)
                    dsl = A[0:tz, 128 * ti:128 * ti + tz]
                    nc.vector.scalar_tensor_tensor(dsl, ident[0:tz, 0:tz],
                                                   dv_all[0:tz, ti:ti + 1], dsl,
                                                   op0=OP.mult, op1=OP.add)
                    A_t.append(A)
                ahatT = []
                for jc in range(3):
                    pst = ps_p.tile([128, SPE], F32, name="ps", tag="ps")
                    for ti, (ts, tz) in enumerate(TOK):
                        tze = tz if tz > 1 else 2
                        nc.tensor.transpose(pst[:, ts:ts + tze],
                                            A_t[ti][0:tze, 128 * jc:128 * jc + 128],
                                            ident[0:tze, 0:tze])
                    ah = ahat_p.tile([128, SPE], F16, name="ahat", tag="ahat")
                    if jc < 2:
                        nc.vector.tensor_scalar_mul(ah[:, :], pst[:, :], dv_all[:, jc:jc + 1])
                    else:
                        nc.gpsimd.tensor_copy(ah[:, :], zcst[:, :])
                        nc.vector.tensor_scalar_mul(ah[0:1, 0:SP], pst[0:1, 0:SP],
                                                    dv_all[0:1, 2:3])
                    ahatT.append(ah)
                return ahatT

            def conv(f_in, widx, ahatT, elu, bset):
                """One GCNConv on all four features. widx 0 -> c1 weights, 1 -> c2."""
                g = {}
                for key in ("ha", "da", "ht", "dt"):
                    w1, w2, chunks, width = wsets[key]
                    W = (w1, w2)[widx]
                    gl = []
                    for ti, (ts, tz) in enumerate(TOK):
                        psg = ps_p.tile([128, width], F32, name="ps", tag="ps")
                        nchunk = len(chunks)
                        for fi, (fs, fz) in enumerate(chunks):
                            nc.tensor.matmul(psg[:, :], f_in[key][fi][:, ts:ts + 128],
                                             W[fi][:, :], start=(fi == 0),
                                             stop=(fi == nchunk - 1))
                        gt = g_p.tile([128, width], F16, name="g", tag="g")
                        nc.vector.tensor_copy(gt[:, :], psg[:, :])
                        gl.append(gt)
                    g[key] = gl
                f_out = new_feats()
                for key, ci, fs, fz, _ in OUTCH:
                    psh = ps_p.tile([fz, SPE], F32, name="ps", tag="ps")
                    for jc in range(3):
                        nc.tensor.matmul(psh[:, :], g[key][jc][:, fs:fs + fz],
                                         ahatT[jc][:, :], start=(jc == 0), stop=(jc == 2))
                    bias_ap = bset[key][ci][:, :]
                    if elu:
                        elu_epilogue(psh[:, 0:SP], bias_ap, f_out[key][ci][0:fz, 0:SP], fz)
                    else:
                        nc.scalar.activation(f_out[key][ci][0:fz, 0:SP], psh[:, 0:SP],
                                             AF.Identity, bias=bias_ap)
                return f_out

            for b in range(BB):
                # ---- load x token-major, transpose to feature-major x^T ----
                xs = []
                for ti, (ts, tz) in enumerate(TOK):
                    xt = xtok_p.tile([max(tz, 2), D], F32, name="xtok", tag="xtok")
                    if ti == 0:
                        nc.sync.dma_start(out=xt[0:1, :], in_=sent_d[:, :])
                        nc.sync.dma_start(out=xt[1:128, :], in_=inp_d[b, 0:127, :])
                    else:
                        nc.sync.dma_start(out=xt[0:tz, :], in_=inp_d[b, ts - 1:ts - 1 + tz, :])
                    xs.append(xt)
                xT = []
                for fc, (fs, fz) in enumerate(KC6):
                    x_f = xT_p.tile([128, SPE], F16, name="xT", tag="xT")
                    for ti, (ts, tz) in enumerate(TOK):
                        tze = tz if tz > 1 else 2
                        pst = ps_p.tile([128, tze], F32, name="ps", tag="ps")
                        nc.tensor.transpose(pst[:, :], xs[ti][0:tze, fs:fs + fz],
                                            ident[0:tze, 0:tze])
                        nc.vector.tensor_copy(x_f[:, ts:ts + tze], pst[:, :])
                    xT.append(x_f)

                # ---- initial projection + elu -> feature-major feats ----
                feats = new_feats()
                for key, ci, fs, fz, base in OUTCH:
                    psp = ps_p.tile([fz, SPE], F32, name="ps", tag="ps")
                    for kc, (ks, kz) in enumerate(KC6):
                        nc.tensor.matmul(psp[:, :], wall[kc][:, base + fs:base + fs + fz],
                                         xT[kc][:, :], start=(kc == 0), stop=(kc == 5))
                    elu_epilogue(psp[:, 0:SP], ballc[(key, ci)][:, :],
                                 feats[key][ci][0:fz, 0:SP], fz)

                # ---- GNN layers ----
                for k in range(LAYERS):
                    lps = bilinear(k, feats)
                    ahatT = build_ahatT(lps)
                    e1 = conv(feats, 0, ahatT, elu=True, bset=c1b)
                    feats = conv(e1, 1, ahatT, elu=False, bset=c2b)

                # ---- final bilinear: logits + u4 (g=500 row term) + bil_b[2] ----
                lps = bilinear(LAYERS, feats)
                u4ps = []
                for ti, (ts, tz) in enumerate(TOK):
                    ps4 = ps_p.tile([128, 2], F32, name="ps", tag="ps")
                    for ei, (es, ez) in enumerate(ARCC):
                        nc.tensor.matmul(ps4[:, :], feats["ha"][ei][:, ts:ts + 128],
                                         w4[ei][:, :], start=(ei == 0), stop=(ei == 3))
                    u4ps.append(ps4)
                for ti, (ts, tz) in enumerate(TOK):
                    u4b = tiny_p.tile([tz, 1], F32, name="tiny", tag="tiny")
                    nc.vector.tensor_add(u4b[:, :], u4ps[ti][0:tz, 0:1], bb2[0:tz, :])
                    ob = ost_p.tile([tz, SP], F32, name="ostage", tag="ostage")
                    nc.scalar.activation(ob[:, :], lps[ti][0:tz, 0:SP], AF.Identity,
                                         bias=u4b[:, :])
                    nc.sync.dma_start(out=arc_d[b, ts:ts + tz, :], in_=ob[:, :])

                # ---- tag outputs: transpose feature-major -> token-major ----
                for key, dram in (("ht", ht_d), ("dt", dt_d)):
                    src = feats[key][0]
                    for ti, (ts, tz) in enumerate(TOK):
                        pst = ps_p.tile([128, TAG], F16, name="ps", tag="ps")
                        nc.tensor.transpose(pst[:, :], src[:, ts:ts + 128],
                                            ident16[0:100, 0:100])
                        ob = ost_p.tile([tz, TAG], F32, name="ostage", tag="ostage")
                        nc.vector.tensor_copy(ob[:, :], pst[0:tz, :])
                        nc.sync.dma_start(out=dram[b, ts:ts + tz, :], in_=ob[:, :])

    nc.finalize()
    return nc


_PROGRAM = None


def kernel(_run_kwargs=None, **inputs):
    global _PROGRAM
    run_kwargs = _run_kwargs or {}
    inp = np.asarray(inputs["inputs"], np.float32)
    sent = np.asarray(inputs["head_sentinel"], np.float32).reshape(1, D)
    wall = np.concatenate(
        [np.asarray(inputs[k], np.float32) for k in ("W_ha", "W_da", "W_ht", "W_dt")], axis=1
    )
    ball = np.concatenate(
        [np.asarray(inputs[k], np.float32) for k in ("b_ha", "b_da", "b_ht", "b_dt")]
    ).reshape(1200, 1)
    bilw = np.asarray(inputs["bil_W"], np.float32)
    bilb = np.asarray(inputs["bil_b"], np.float32)
    w4 = np.zeros((ARC, 2), np.float32)
    w4[:, 0] = bilw[LAYERS, 0:ARC, ARC]
    params = {
        "sent": sent, "wall": wall, "ball": ball, "bilw": bilw, "w4": w4,
        "c1aw": np.asarray(inputs["c1a_W"], np.float32),
        "c2aw": np.asarray(inputs["c2a_W"], np.float32),
        "c1rw": np.asarray(inputs["c1r_W"], np.float32),
        "c2rw": np.asarray(inputs["c2r_W"], np.float32),
        "c1ab": np.asarray(inputs["c1a_b"], np.float32).reshape(ARC, 1),
        "c2ab": np.asarray(inputs["c2a_b"], np.float32).reshape(ARC, 1),
        "c1rb": np.asarray(inputs["c1r_b"], np.float32).reshape(TAG, 1),
        "c2rb": np.asarray(inputs["c2r_b"], np.float32).reshape(TAG, 1),
        "bb2": np.full((128, 1), bilb[LAYERS] + bilw[LAYERS, ARC, ARC], np.float32),
        "bwbc": np.ascontiguousarray(bilw[:, ARC, :, None]),
    }

    if _PROGRAM is None:
        _PROGRAM = build_program()
    nc = _PROGRAM

    in_maps = [dict(params, inp=np.ascontiguousarray(inp[c * BB:(c + 1) * BB]))
               for c in range(NCORES)]
    res = run_bass_kernel_spmd(nc, in_maps, list(range(NCORES)), **run_kwargs)
    arc = np.concatenate([r["arc"] for r in res.results], axis=0)
    ht = np.concatenate([r["ht"] for r in res.results], axis=0)
    dt = np.concatenate([r["dt"] for r in res.results], axis=0)
    if run_kwargs:
        return (arc, ht, dt), res
    return arc, ht, dt
